# revision 23
# baseline (speedup 1.0000x reference)
"""Distributed GQA decode-attention kernel for 8 TRN2 NeuronCores.

Problem: B=32 batch of single-token decode against a 4096-entry KV cache.
  - fused QKV projection (wqkv [6144,4096]) with RoPE at position ctx_len
    folded into the weight matrix on host (rotation is linear, position is
    host-known), q rows pre-scaled by 1/sqrt(HD)
  - GQA attention: 32 q heads, 8 kv heads -> tensor-parallel over kv head
    groups: core g owns q heads [4g..4g+4), kv head g, cache slice g
  - output projection wo columns sharded by head group, AllReduce the
    partial outputs.

Device dataflow per core (all PE compute in bf16 with f32 PSUM accum):
  1. xqkv[32,768] = x @ wqkv_g.T    (32 k-tiles of 128)
  2. PE-transpose q -> qT [128d, 4h] per batch; kT_new [128d, 32b]
  3. per batch b: DMA K^T[b] [128d, 4096t] (host pre-transposed, bf16) and
     V'[b] [128p, 32tt, 129] (host packed, d padded with a ones column);
     insert k_new col / v_new row at position ctx_len;
     scores^T tile [128t, 4h] x 32 t-tiles = one psum [128, 128];
     exp on ACT -> w^T bf16; o_psum[4, 129] += w^T_tile.T @ V'_tile
     (col 128 accumulates sum(w) via the ones column); normalize by
     reciprocal; PE-transpose o -> oT [128, 32b] per head group.
  4. out_partial[32, 4096] = oT.T @ woT_g  (4 k-tiles x 8 n-chunks)
  5. AllReduce(out_partial) over 8 cores.
"""

import sys

sys.path.insert(0, "/opt/trn_rl_repo")

import numpy as np
import ml_dtypes

B = 32
DIM = 4096
CACHE_LEN = 4096
HD = 128
N_HEADS = 32
N_KV = 8
N_CORES = 8
HPG = N_HEADS // N_KV  # 4 query heads per kv group
QKV_G = HPG * HD + 2 * HD  # 768 qkv output rows per core
ROPE_THETA = 10000.0

BF16 = ml_dtypes.bfloat16

_MODULE_CACHE = {}


def _build_module(pos: int):
    from concourse import bacc, bass, tile, mybir, masks

    f32 = mybir.dt.float32
    bf16 = mybir.dt.bfloat16
    PSUM = bass.MemorySpace.PSUM

    nc = bacc.Bacc("TRN2", target_bir_lowering=False, debug=False, num_devices=N_CORES)

    xT_d = nc.dram_tensor("xT", [128, 32 * B], bf16, kind="ExternalInput")
    wqkvT_d = nc.dram_tensor("wqkvT", [DIM, QKV_G], bf16, kind="ExternalInput")
    kT_d = nc.dram_tensor("kT", [B, HD, CACHE_LEN], bf16, kind="ExternalInput")
    vp_d = nc.dram_tensor("vp", [B, 128, 32, 129], bf16, kind="ExternalInput")
    woT_d = nc.dram_tensor("woT", [HPG * HD, DIM], bf16, kind="ExternalInput")
    out_d = nc.dram_tensor("out", [2 * (B // 2 // N_CORES), DIM], bf16, kind="ExternalOutput")

    tt_pos, p_pos = pos // 128, pos % 128  # new-token slot in the V' packing

    with tile.TileContext(nc) as tc:
        with (
            tc.tile_pool(name="const", bufs=1) as cpool,
            tc.tile_pool(name="persist", bufs=1) as ppool,
            tc.tile_pool(name="kcache", bufs=3) as kpool,
            tc.tile_pool(name="vcache", bufs=4) as vpool,
            tc.tile_pool(name="sm", bufs=3) as smpool,
            tc.tile_pool(name="small", bufs=4) as spool,
            tc.tile_pool(name="spsum", bufs=2, space=PSUM) as sppool,
            tc.tile_pool(name="opsum", bufs=2, space=PSUM) as oppool,
            tc.tile_pool(name="tpsum", bufs=2, space=PSUM) as tppool,
            tc.tile_pool(name="wo", bufs=1) as wopool,
            tc.tile_pool(name="dram", bufs=1, space="DRAM") as dpool,
        ):
            ident = cpool.tile([128, 128], bf16)
            masks.make_identity(nc, ident[:])

            # ---------------- phase 1: fused QKV projection ----------------
            xT_sb = ppool.tile([128, 32 * B], bf16)  # [p, (kt b)]
            nc.scalar.dma_start(xT_sb[:], xT_d[:])

            # tiny warmup collective: absorbs the first-call setup cost and
            # cross-core skew so the real ReduceScatters below run fast
            warm_in = dpool.tile([N_CORES, 64], bf16, name="warm_in")
            warm_out = dpool.tile([1, 64], bf16, name="warm_out")
            nc.gpsimd.dma_start(warm_in[:], xT_sb[0:N_CORES, 0:64])
            nc.gpsimd.collective_compute(
                "ReduceScatter",
                mybir.AluOpType.add,
                ins=[warm_in.opt()],
                outs=[warm_out.opt()],
                replica_groups=[list(range(N_CORES))],
            )

            xqkv_sb = ppool.tile([B, QKV_G], bf16)
            qT_sb = ppool.tile([128, B * HPG], bf16)  # [d, (b h)]
            kTn_sb = ppool.tile([128, B], bf16)  # new k token, [d, b]

            with (
                tc.tile_pool(name="wq", bufs=8) as wqpool,
                tc.tile_pool(name="xpsum", bufs=1, space=PSUM) as xppool,
            ):
                xps_a = xppool.tile([B, 512], f32)
                xps_b = xppool.tile([B, 256], f32)
                for kt in range(32):
                    wt = wqpool.tile([128, QKV_G], bf16)
                    nc.scalar.dma_start(wt[:], wqkvT_d[kt * 128 : (kt + 1) * 128, :])
                    lhsT = xT_sb[:, kt * 32 : (kt + 1) * 32]
                    nc.tensor.matmul(
                        xps_a[:], lhsT, wt[:, 0:512], start=(kt == 0), stop=(kt == 31)
                    )
                    nc.tensor.matmul(
                        xps_b[:], lhsT, wt[:, 512:768], start=(kt == 0), stop=(kt == 31)
                    )
                nc.vector.tensor_copy(xqkv_sb[:, 0:512], xps_a[:])
                nc.vector.tensor_copy(xqkv_sb[:, 512:768], xps_b[:])

            # transpose q heads and k_new: [32b, 128d] -> [128d, 32b]
            qT_view = qT_sb[:].rearrange("p (b h) -> p b h", h=HPG)
            for h in range(HPG):
                tp = tppool.tile([128, B], bf16, tag="tp")
                nc.tensor.transpose(
                    tp[:], xqkv_sb[:, h * HD : (h + 1) * HD], ident[0:B, 0:B]
                )
                nc.vector.tensor_copy(qT_view[:, :, h], tp[:])
            tpk = tppool.tile([128, B], bf16, tag="tp")
            nc.tensor.transpose(
                tpk[:], xqkv_sb[:, HPG * HD : HPG * HD + HD], ident[0:B, 0:B]
            )
            nc.vector.tensor_copy(kTn_sb[:], tpk[:])

            # ------- phase 2+3: attention + split output proj + overlapped RS -------
            woppool_cm = tc.tile_pool(name="wops", bufs=2, space=PSUM)
            woppool = woppool_cm.__enter__()
            oT_sb = ppool.tile([128, HPG * B], bf16)  # [d, (h b)]
            oT_view = oT_sb[:].rearrange("p (h b) -> p h b", b=B)

            wo_t = []
            for h in range(HPG):
                t = wopool.tile([128, DIM], bf16, tag=f"wo{h}")
                nc.scalar.dma_start(t[:], woT_d[h * 128 : (h + 1) * 128, :])
                wo_t.append(t)

            # batch segments: a ReduceScatter fires after each, overlapping the
            # next segment's attention; only the last (small) one is a tail
            SEGS = [(0, 16), (16, 24), (24, 32)]
            nseg = len(SEGS)
            out_hb = [
                ppool.tile([16, DIM], bf16, name=f"oh{i}", tag="oh", bufs=2)
                for i, (s0, s1) in enumerate(SEGS)
            ]
            cc_in = [
                dpool.tile([s1 - s0, DIM], bf16, name=f"ci{i}", tag=f"ci{i}")
                for i, (s0, s1) in enumerate(SEGS)
            ]
            cc_out = [
                dpool.tile([(s1 - s0) // N_CORES, DIM], bf16, name=f"co{i}", tag=f"co{i}")
                for i, (s0, s1) in enumerate(SEGS)
            ]

            for seg, (s0, s1) in enumerate(SEGS):
                for bi in range(0, s1 - s0, 2):  # 2-batch DMA groups
                    b0 = s0 + bi
                    kgrp = kpool.tile([128, 2 * CACHE_LEN], bf16)
                    nc.sync.dma_start(
                        kgrp[:].rearrange("p (bb t) -> p bb t", bb=2),
                        kT_d[:][b0 : b0 + 2].rearrange("bb p t -> p bb t"),
                    )
                    vgrp = vpool.tile([128, 2 * 32 * 129], bf16)
                    nc.sync.dma_start(
                        vgrp[:].rearrange("p (bb x) -> p bb x", bb=2),
                        vp_d[:][b0 : b0 + 2].rearrange("bb p t d -> p bb (t d)"),
                    )
                    for i in range(2):
                        b = b0 + i
                        ktile = kgrp[:, i * CACHE_LEN : (i + 1) * CACHE_LEN]
                        vtile = vgrp[:, i * 4128 : (i + 1) * 4128]
                        # overwrite cache column at `pos` with the new roped k
                        nc.vector.tensor_copy(
                            ktile[:, pos : pos + 1], kTn_sb[:, b : b + 1]
                        )
                        # overwrite the v row for the new token (partition shift)
                        nc.sync.dma_start(
                            vtile[
                                p_pos : p_pos + 1,
                                tt_pos * 129 : tt_pos * 129 + 128,
                            ],
                            xqkv_sb[b : b + 1, HPG * HD + HD : QKV_G],
                        )

                        sps = sppool.tile([128, 128], f32)
                        rhs_q = qT_sb[:, b * HPG : (b + 1) * HPG]
                        for tt in range(32):
                            nc.tensor.matmul(
                                sps[:, tt * HPG : (tt + 1) * HPG],
                                ktile[:, tt * 128 : (tt + 1) * 128],
                                rhs_q,
                                start=True,
                                stop=True,
                            )
                        ew = smpool.tile([128, 128], bf16)
                        nc.scalar.activation(
                            ew[:], sps[:], mybir.ActivationFunctionType.Exp
                        )
                        # mask cache slots beyond ctx_len (none when pos == 4095)
                        if tt_pos < 31:
                            nc.vector.memset(ew[:, (tt_pos + 1) * HPG : 128], 0.0)
                        if p_pos < 127:
                            nc.vector.memset(
                                ew[p_pos + 1 : 128, tt_pos * HPG : (tt_pos + 1) * HPG],
                                0.0,
                            )

                        ops = oppool.tile([HPG, 132], f32)
                        for tt in range(32):
                            nc.tensor.matmul(
                                ops[:, 0:129],
                                ew[:, tt * HPG : (tt + 1) * HPG],
                                vtile[:, tt * 129 : (tt + 1) * 129],
                                start=(tt == 0),
                                stop=(tt == 31),
                            )
                        recip = spool.tile([HPG, 1], f32)
                        nc.vector.reciprocal(recip[:], ops[:, 128:129])
                        onorm = spool.tile([HPG, HD], bf16)
                        nc.vector.tensor_scalar_mul(onorm[:], ops[:, 0:128], recip[:])

                        tpo = tppool.tile([128, B], bf16, tag="tp")
                        nc.tensor.transpose(
                            tpo[:, 0:HPG], onorm[:], ident[0:HPG, 0:HPG]
                        )
                        nc.vector.tensor_copy(oT_view[:, :, b], tpo[:, 0:HPG])

                # output projection for this segment, then ReduceScatter it
                # while the next segment's attention keeps streaming
                for j in range(8):
                    wps = woppool.tile([16, 512], f32, name="wps", tag="wps")
                    for h in range(HPG):
                        nc.tensor.matmul(
                            wps[0 : s1 - s0, :],
                            oT_view[:, h, s0:s1],
                            wo_t[h][:, j * 512 : (j + 1) * 512],
                            start=(h == 0),
                            stop=(h == HPG - 1),
                        )
                    nc.vector.tensor_copy(
                        out_hb[seg][0 : s1 - s0, j * 512 : (j + 1) * 512],
                        wps[0 : s1 - s0, :],
                    )
                nc.gpsimd.dma_start(cc_in[seg][:], out_hb[seg][0 : s1 - s0, :])
                nc.gpsimd.collective_compute(
                    "ReduceScatter",
                    mybir.AluOpType.add,
                    ins=[cc_in[seg].opt()],
                    outs=[cc_out[seg].opt()],
                    replica_groups=[list(range(N_CORES))],
                )
                nc.gpsimd.dma_start(
                    out_d[s0 // N_CORES : s1 // N_CORES, :], cc_out[seg][:]
                )
            woppool_cm.__exit__(None, None, None)

    nc.compile()
    return nc


def _host_prep(x, cache_k, cache_v, wqkv, wo, pos):
    """Shard + lay out inputs per core. Returns in_maps (list of 8 dicts)."""
    x = np.asarray(x, np.float32)
    wqkv = np.asarray(wqkv, np.float32)
    wo = np.asarray(wo, np.float32)
    cache_k = np.asarray(cache_k)
    cache_v = np.asarray(cache_v)

    # fold RoPE at position `pos` (+ 1/sqrt(HD) for q) into wqkv rows
    freqs = ROPE_THETA ** (-np.arange(0, HD, 2, dtype=np.float64) / HD)
    ang = pos * freqs
    c, s = np.cos(ang)[None, :, None], np.sin(ang)[None, :, None]

    def rot(wr):  # [nh, HD, DIM]
        w1, w2 = wr[:, 0::2, :], wr[:, 1::2, :]
        out = np.empty_like(wr)
        out[:, 0::2, :] = c * w1 - s * w2
        out[:, 1::2, :] = s * w1 + c * w2
        return out

    nq, nk = N_HEADS * HD, N_KV * HD
    w64 = wqkv.astype(np.float64)
    wq = rot(w64[:nq].reshape(N_HEADS, HD, DIM)) / np.sqrt(HD)
    wk = rot(w64[nq : nq + nk].reshape(N_KV, HD, DIM))
    wv = w64[nq + nk :].reshape(N_KV, HD, DIM)

    # xT packed to SBUF layout [p, (kt b)]
    xT = np.ascontiguousarray(
        x.T.reshape(32, 128, B).transpose(1, 0, 2).reshape(128, 32 * B)
    ).astype(BF16)

    woT = wo.T  # [DIM, DIM]

    in_maps = []
    for g in range(N_CORES):
        wsh = np.concatenate(
            [wq[g * HPG : (g + 1) * HPG].reshape(-1, DIM), wk[g], wv[g]], axis=0
        )  # [768, DIM]
        wqkvT = np.ascontiguousarray(wsh.T).astype(BF16)  # [DIM, 768]

        kT = np.ascontiguousarray(
            cache_k[:, :, g, :].transpose(0, 2, 1)
        ).astype(BF16)  # [B, HD, T]

        vsh = cache_v[:, :, g, :].reshape(B, 32, 128, HD).transpose(0, 2, 1, 3)
        vp = np.ones((B, 128, 32, 129), BF16)
        vp[:, :, :, :128] = vsh.astype(BF16)  # ones column at d=128

        wo_sh = np.ascontiguousarray(
            woT[g * HPG * HD : (g + 1) * HPG * HD, :]
        ).astype(BF16)  # [512, DIM]

        in_maps.append(
            {"xT": xT, "wqkvT": wqkvT, "kT": kT, "vp": vp, "woT": wo_sh}
        )
    return in_maps


def kernel(x, cache_k, cache_v, wqkv, wo, ctx_len):
    from concourse import bass_utils

    pos = int(ctx_len)
    assert 0 <= pos < CACHE_LEN

    if pos not in _MODULE_CACHE:
        _MODULE_CACHE[pos] = _build_module(pos)
    nc = _MODULE_CACHE[pos]

    in_maps = _host_prep(x, cache_k, cache_v, wqkv, wo, pos)
    res = bass_utils.run_bass_kernel_spmd(
        nc, in_maps, core_ids=list(range(N_CORES))
    )
    out = np.empty((B, DIM), np.float32)
    segs = [(0, 16), (16, 24), (24, 32)]
    for r in range(N_CORES):
        blk = np.asarray(res.results[r]["out"]).astype(np.float32)
        for s0, s1 in segs:
            rpc = (s1 - s0) // N_CORES
            out[s0 + r * rpc : s0 + (r + 1) * rpc] = blk[s0 // N_CORES : s0 // N_CORES + rpc]
    return out


if __name__ == "__main__":
    import reference

    inputs = reference.setup_inputs()
    out = kernel(**{k: np.asarray(v) for k, v in inputs.items()})
    ref = np.asarray(reference.reference(**inputs))
    err = np.linalg.norm(out - ref) / np.linalg.norm(ref)
    print("Relative error:", err)


# revision 24
# speedup vs baseline: 1.0717x; 1.0717x over previous
"""Distributed GQA decode-attention kernel for 8 TRN2 NeuronCores.

Problem: B=32 batch of single-token decode against a 4096-entry KV cache.
  - fused QKV projection (wqkv [6144,4096]) with RoPE at position ctx_len
    folded into the weight matrix on host (rotation is linear, position is
    host-known), q rows pre-scaled by 1/sqrt(HD)
  - GQA attention: 32 q heads, 8 kv heads -> tensor-parallel over kv head
    groups: core g owns q heads [4g..4g+4), kv head g, cache slice g
  - output projection wo columns sharded by head group, AllReduce the
    partial outputs.

Device dataflow per core (all PE compute in bf16 with f32 PSUM accum):
  1. xqkv[32,768] = x @ wqkv_g.T    (32 k-tiles of 128)
  2. PE-transpose q -> qT [128d, 4h] per batch; kT_new [128d, 32b]
  3. per batch b: DMA K^T[b] [128d, 4096t] (host pre-transposed, bf16) and
     V'[b] [128p, 32tt, 129] (host packed, d padded with a ones column);
     insert k_new col / v_new row at position ctx_len;
     scores^T tile [128t, 4h] x 32 t-tiles = one psum [128, 128];
     exp on ACT -> w^T bf16; o_psum[4, 129] += w^T_tile.T @ V'_tile
     (col 128 accumulates sum(w) via the ones column); normalize by
     reciprocal; PE-transpose o -> oT [128, 32b] per head group.
  4. out_partial[32, 4096] = oT.T @ woT_g  (4 k-tiles x 8 n-chunks)
  5. AllReduce(out_partial) over 8 cores.
"""

import sys

sys.path.insert(0, "/opt/trn_rl_repo")

import numpy as np
import ml_dtypes

B = 32
DIM = 4096
CACHE_LEN = 4096
HD = 128
N_HEADS = 32
N_KV = 8
N_CORES = 8
HPG = N_HEADS // N_KV  # 4 query heads per kv group
QKV_G = HPG * HD + 2 * HD  # 768 qkv output rows per core
ROPE_THETA = 10000.0

BF16 = ml_dtypes.bfloat16

_MODULE_CACHE = {}


def _build_module(pos: int):
    from concourse import bacc, bass, tile, mybir, masks

    f32 = mybir.dt.float32
    bf16 = mybir.dt.bfloat16
    PSUM = bass.MemorySpace.PSUM

    nc = bacc.Bacc("TRN2", target_bir_lowering=False, debug=False, num_devices=N_CORES)

    xT_d = nc.dram_tensor("xT", [128, 32 * B], bf16, kind="ExternalInput")
    wqkvT_d = nc.dram_tensor("wqkvT", [DIM, QKV_G], bf16, kind="ExternalInput")
    kT_d = nc.dram_tensor("kT", [B, HD, CACHE_LEN], bf16, kind="ExternalInput")
    vp_d = nc.dram_tensor("vp", [B, 128, 32, 129], bf16, kind="ExternalInput")
    woT_d = nc.dram_tensor("woT", [HPG * HD, DIM], bf16, kind="ExternalInput")
    out_d = nc.dram_tensor("out", [2 * (B // 2 // N_CORES), DIM], bf16, kind="ExternalOutput")

    tt_pos, p_pos = pos // 128, pos % 128  # new-token slot in the V' packing

    with tile.TileContext(nc) as tc:
        with (
            tc.tile_pool(name="const", bufs=1) as cpool,
            tc.tile_pool(name="persist", bufs=1) as ppool,
            tc.tile_pool(name="kcache", bufs=3) as kpool,
            tc.tile_pool(name="vcache", bufs=3) as vpool,
            tc.tile_pool(name="sm", bufs=3) as smpool,
            tc.tile_pool(name="small", bufs=4) as spool,
            tc.tile_pool(name="spsum", bufs=2, space=PSUM) as sppool,
            tc.tile_pool(name="opsum", bufs=2, space=PSUM) as oppool,
            tc.tile_pool(name="tpsum", bufs=2, space=PSUM) as tppool,
            tc.tile_pool(name="wo", bufs=1) as wopool,
            tc.tile_pool(name="dram", bufs=1, space="DRAM") as dpool,
        ):
            ident = cpool.tile([128, 128], bf16)
            masks.make_identity(nc, ident[:])

            # ---------------- phase 1: fused QKV projection ----------------
            xT_sb = ppool.tile([128, 32 * B], bf16)  # [p, (kt b)]
            nc.scalar.dma_start(xT_sb[:], xT_d[:])

            # tiny warmup collective: absorbs the first-call setup cost and
            # cross-core skew so the real ReduceScatters below run fast
            warm_in = dpool.tile([N_CORES, 64], bf16, name="warm_in")
            warm_out = dpool.tile([1, 64], bf16, name="warm_out")
            nc.gpsimd.dma_start(warm_in[:], xT_sb[0:N_CORES, 0:64])
            nc.gpsimd.collective_compute(
                "ReduceScatter",
                mybir.AluOpType.add,
                ins=[warm_in.opt()],
                outs=[warm_out.opt()],
                replica_groups=[list(range(N_CORES))],
            )

            xqkv_sb = ppool.tile([B, QKV_G], bf16)
            qT_sb = ppool.tile([128, B * HPG], bf16)  # [d, (b h)]
            kTn_sb = ppool.tile([128, B], bf16)  # new k token, [d, b]

            with (
                tc.tile_pool(name="wq", bufs=8) as wqpool,
                tc.tile_pool(name="xpsum", bufs=1, space=PSUM) as xppool,
            ):
                xps_a = xppool.tile([B, 512], f32)
                xps_b = xppool.tile([B, 256], f32)
                for kt in range(32):
                    wt = wqpool.tile([128, QKV_G], bf16)
                    nc.scalar.dma_start(wt[:], wqkvT_d[kt * 128 : (kt + 1) * 128, :])
                    lhsT = xT_sb[:, kt * 32 : (kt + 1) * 32]
                    nc.tensor.matmul(
                        xps_a[:], lhsT, wt[:, 0:512], start=(kt == 0), stop=(kt == 31)
                    )
                    nc.tensor.matmul(
                        xps_b[:], lhsT, wt[:, 512:768], start=(kt == 0), stop=(kt == 31)
                    )
                nc.vector.tensor_copy(xqkv_sb[:, 0:512], xps_a[:])
                nc.vector.tensor_copy(xqkv_sb[:, 512:768], xps_b[:])

            # transpose q heads and k_new: [32b, 128d] -> [128d, 32b]
            qT_view = qT_sb[:].rearrange("p (b h) -> p b h", h=HPG)
            for h in range(HPG):
                tp = tppool.tile([128, B], bf16, tag="tp")
                nc.tensor.transpose(
                    tp[:], xqkv_sb[:, h * HD : (h + 1) * HD], ident[0:B, 0:B]
                )
                nc.vector.tensor_copy(qT_view[:, :, h], tp[:])
            tpk = tppool.tile([128, B], bf16, tag="tp")
            nc.tensor.transpose(
                tpk[:], xqkv_sb[:, HPG * HD : HPG * HD + HD], ident[0:B, 0:B]
            )
            nc.vector.tensor_copy(kTn_sb[:], tpk[:])

            # ------- phase 2+3: attention + split output proj + overlapped RS -------
            woppool_cm = tc.tile_pool(name="wops", bufs=2, space=PSUM)
            woppool = woppool_cm.__enter__()
            oT_sb = ppool.tile([128, HPG * B], bf16)  # [d, (h b)]
            oT_view = oT_sb[:].rearrange("p (h b) -> p h b", b=B)

            wo_t = []
            for h in range(HPG):
                t = wopool.tile([128, DIM], bf16, tag=f"wo{h}")
                nc.scalar.dma_start(t[:], woT_d[h * 128 : (h + 1) * 128, :])
                wo_t.append(t)

            # batch segments: a ReduceScatter fires after each, overlapping the
            # next segment's attention; only the last (small) one is a tail
            SEGS = [(0, 16), (16, 24), (24, 32)]
            nseg = len(SEGS)
            out_hb = [
                ppool.tile([s1 - s0, DIM], bf16, name=f"oh{i}", tag=f"oh{i}")
                for i, (s0, s1) in enumerate(SEGS)
            ]
            cc_in = [
                dpool.tile([s1 - s0, DIM], bf16, name=f"ci{i}", tag=f"ci{i}")
                for i, (s0, s1) in enumerate(SEGS)
            ]
            cc_out = [
                dpool.tile([(s1 - s0) // N_CORES, DIM], bf16, name=f"co{i}", tag=f"co{i}")
                for i, (s0, s1) in enumerate(SEGS)
            ]

            for seg, (s0, s1) in enumerate(SEGS):
                for bi in range(0, s1 - s0, 2):  # 2-batch DMA groups
                    b0 = s0 + bi
                    kgrp = kpool.tile([128, 2 * CACHE_LEN], bf16)
                    nc.sync.dma_start(
                        kgrp[:].rearrange("p (bb t) -> p bb t", bb=2),
                        kT_d[:][b0 : b0 + 2].rearrange("bb p t -> p bb t"),
                    )
                    vgrp = vpool.tile([128, 2 * 32 * 129], bf16)
                    nc.sync.dma_start(
                        vgrp[:].rearrange("p (bb x) -> p bb x", bb=2),
                        vp_d[:][b0 : b0 + 2].rearrange("bb p t d -> p bb (t d)"),
                    )
                    for i in range(2):
                        b = b0 + i
                        ktile = kgrp[:, i * CACHE_LEN : (i + 1) * CACHE_LEN]
                        vtile = vgrp[:, i * 4128 : (i + 1) * 4128]
                        # overwrite cache column at `pos` with the new roped k
                        nc.vector.tensor_copy(
                            ktile[:, pos : pos + 1], kTn_sb[:, b : b + 1]
                        )
                        # overwrite the v row for the new token (partition shift)
                        nc.sync.dma_start(
                            vtile[
                                p_pos : p_pos + 1,
                                tt_pos * 129 : tt_pos * 129 + 128,
                            ],
                            xqkv_sb[b : b + 1, HPG * HD + HD : QKV_G],
                        )

                        sps = sppool.tile([128, 128], f32)
                        rhs_q = qT_sb[:, b * HPG : (b + 1) * HPG]
                        for tt in range(32):
                            nc.tensor.matmul(
                                sps[:, tt * HPG : (tt + 1) * HPG],
                                ktile[:, tt * 128 : (tt + 1) * 128],
                                rhs_q,
                                start=True,
                                stop=True,
                            )
                        ew = smpool.tile([128, 128], bf16)
                        nc.scalar.activation(
                            ew[:], sps[:], mybir.ActivationFunctionType.Exp
                        )
                        # mask cache slots beyond ctx_len (none when pos == 4095)
                        if tt_pos < 31:
                            nc.vector.memset(ew[:, (tt_pos + 1) * HPG : 128], 0.0)
                        if p_pos < 127:
                            nc.vector.memset(
                                ew[p_pos + 1 : 128, tt_pos * HPG : (tt_pos + 1) * HPG],
                                0.0,
                            )

                        ops = oppool.tile([HPG, 132], f32)
                        for tt in range(32):
                            nc.tensor.matmul(
                                ops[:, 0:129],
                                ew[:, tt * HPG : (tt + 1) * HPG],
                                vtile[:, tt * 129 : (tt + 1) * 129],
                                start=(tt == 0),
                                stop=(tt == 31),
                            )
                        recip = spool.tile([HPG, 1], f32)
                        nc.vector.reciprocal(recip[:], ops[:, 128:129])
                        onorm = spool.tile([HPG, HD], bf16)
                        nc.vector.tensor_scalar_mul(onorm[:], ops[:, 0:128], recip[:])

                        tpo = tppool.tile([128, B], bf16, tag="tp")
                        nc.tensor.transpose(
                            tpo[:, 0:HPG], onorm[:], ident[0:HPG, 0:HPG]
                        )
                        nc.vector.tensor_copy(oT_view[:, :, b], tpo[:, 0:HPG])

                # output projection for this segment, then ReduceScatter it
                # while the next segment's attention keeps streaming
                for j in range(8):
                    wps = woppool.tile([16, 512], f32, name="wps", tag="wps")
                    for h in range(HPG):
                        nc.tensor.matmul(
                            wps[0 : s1 - s0, :],
                            oT_view[:, h, s0:s1],
                            wo_t[h][:, j * 512 : (j + 1) * 512],
                            start=(h == 0),
                            stop=(h == HPG - 1),
                        )
                    nc.vector.tensor_copy(
                        out_hb[seg][:, j * 512 : (j + 1) * 512], wps[0 : s1 - s0, :]
                    )
                nc.gpsimd.dma_start(cc_in[seg][:], out_hb[seg][:])
                nc.gpsimd.collective_compute(
                    "ReduceScatter",
                    mybir.AluOpType.add,
                    ins=[cc_in[seg].opt()],
                    outs=[cc_out[seg].opt()],
                    replica_groups=[list(range(N_CORES))],
                )
                nc.gpsimd.dma_start(
                    out_d[s0 // N_CORES : s1 // N_CORES, :], cc_out[seg][:]
                )
            woppool_cm.__exit__(None, None, None)

    nc.compile()
    return nc


def _host_prep(x, cache_k, cache_v, wqkv, wo, pos):
    """Shard + lay out inputs per core. Returns in_maps (list of 8 dicts)."""
    x = np.asarray(x, np.float32)
    wqkv = np.asarray(wqkv, np.float32)
    wo = np.asarray(wo, np.float32)
    cache_k = np.asarray(cache_k)
    cache_v = np.asarray(cache_v)

    # fold RoPE at position `pos` (+ 1/sqrt(HD) for q) into wqkv rows
    freqs = ROPE_THETA ** (-np.arange(0, HD, 2, dtype=np.float64) / HD)
    ang = pos * freqs
    c, s = np.cos(ang)[None, :, None], np.sin(ang)[None, :, None]

    def rot(wr):  # [nh, HD, DIM]
        w1, w2 = wr[:, 0::2, :], wr[:, 1::2, :]
        out = np.empty_like(wr)
        out[:, 0::2, :] = c * w1 - s * w2
        out[:, 1::2, :] = s * w1 + c * w2
        return out

    nq, nk = N_HEADS * HD, N_KV * HD
    w64 = wqkv.astype(np.float64)
    wq = rot(w64[:nq].reshape(N_HEADS, HD, DIM)) / np.sqrt(HD)
    wk = rot(w64[nq : nq + nk].reshape(N_KV, HD, DIM))
    wv = w64[nq + nk :].reshape(N_KV, HD, DIM)

    # xT packed to SBUF layout [p, (kt b)]
    xT = np.ascontiguousarray(
        x.T.reshape(32, 128, B).transpose(1, 0, 2).reshape(128, 32 * B)
    ).astype(BF16)

    woT = wo.T  # [DIM, DIM]

    in_maps = []
    for g in range(N_CORES):
        wsh = np.concatenate(
            [wq[g * HPG : (g + 1) * HPG].reshape(-1, DIM), wk[g], wv[g]], axis=0
        )  # [768, DIM]
        wqkvT = np.ascontiguousarray(wsh.T).astype(BF16)  # [DIM, 768]

        kT = np.ascontiguousarray(
            cache_k[:, :, g, :].transpose(0, 2, 1)
        ).astype(BF16)  # [B, HD, T]

        vsh = cache_v[:, :, g, :].reshape(B, 32, 128, HD).transpose(0, 2, 1, 3)
        vp = np.ones((B, 128, 32, 129), BF16)
        vp[:, :, :, :128] = vsh.astype(BF16)  # ones column at d=128

        wo_sh = np.ascontiguousarray(
            woT[g * HPG * HD : (g + 1) * HPG * HD, :]
        ).astype(BF16)  # [512, DIM]

        in_maps.append(
            {"xT": xT, "wqkvT": wqkvT, "kT": kT, "vp": vp, "woT": wo_sh}
        )
    return in_maps


def kernel(x, cache_k, cache_v, wqkv, wo, ctx_len):
    from concourse import bass_utils

    pos = int(ctx_len)
    assert 0 <= pos < CACHE_LEN

    if pos not in _MODULE_CACHE:
        _MODULE_CACHE[pos] = _build_module(pos)
    nc = _MODULE_CACHE[pos]

    in_maps = _host_prep(x, cache_k, cache_v, wqkv, wo, pos)
    res = bass_utils.run_bass_kernel_spmd(
        nc, in_maps, core_ids=list(range(N_CORES))
    )
    out = np.empty((B, DIM), np.float32)
    segs = [(0, 16), (16, 24), (24, 32)]
    for r in range(N_CORES):
        blk = np.asarray(res.results[r]["out"]).astype(np.float32)
        for s0, s1 in segs:
            rpc = (s1 - s0) // N_CORES
            out[s0 + r * rpc : s0 + (r + 1) * rpc] = blk[s0 // N_CORES : s0 // N_CORES + rpc]
    return out


if __name__ == "__main__":
    import reference

    inputs = reference.setup_inputs()
    out = kernel(**{k: np.asarray(v) for k, v in inputs.items()})
    ref = np.asarray(reference.reference(**inputs))
    err = np.linalg.norm(out - ref) / np.linalg.norm(ref)
    print("Relative error:", err)
